# revision 4
# baseline (speedup 1.0000x reference)
"""Trainium2 Bass kernel for the SOCNet battery state-of-charge model.

Math (per battery cell b, timestep t):
    h   = softplus(w0*I + w1*Temp + b1e)
    f   = eta0*(1 + w2e*h + b2e) * I / (3600*Q)
    out[b, 0] = SOC_init(b)          (tiny net on first-timestep features)
    out[b, t] = SOC_init(b) + sum_{j<t} (ts[j+1]-ts[j]) * f[j]

Strategy: pure data parallel over 8 NeuronCores (128 batch rows per core =
128 SBUF partitions).  The tiny per-cell scalars (SOC_init, q1, q2) are
precomputed on host.  The kernel is HBM-traffic-bound, so the streamed
inputs are staged as: time column f32 (dt needs full precision), I/Temp
columns fp8-e4m3 (they only perturb the tiny per-step increments; fp8
noise is ~1e-3 of the output), and the SOC output is written bf16 and
upcast on host (scan state stays fp32 in hardware; bf16 only rounds each
stored element).  That cuts per-core traffic 21 MB -> 8.4 MB.

Per chunk the engines split the elementwise work:
    DVE    : pre-act stt, affine (q2*h+q1), increment mul (+row-sum), scan
    ACT    : exp, ln  (softplus; native Softplus fails to lower)
    GpSimd : dt diff, dt*I
The f32 carry for chunk k+1 is chained via the exact accum_out row sum of
chunk k's increments, so bf16 output rounding never enters the recurrence.
"""

import numpy as np

B, T, F = 1024, 8192, 4
NCORES = 8
BS = B // NCORES  # 128 rows per core == SBUF partition count
TC = 2048         # timesteps per chunk


def _softplus64(x):
    x = x.astype(np.float64)
    return np.logaddexp(0.0, x)


def _build_program(k_piv, piv_is_I, act_scale, reps=1):
    from contextlib import ExitStack

    import bass_rust as _bass_rust
    import concourse.bass as bass
    import concourse.mybir as mybir
    import concourse.tile as tile

    f32 = mybir.dt.float32
    f8 = mybir.dt.float8e4
    bf16 = mybir.dt.bfloat16
    nc = bass.Bass()

    td = nc.dram_tensor("t", [BS, T], f32, kind="ExternalInput")
    id_ = nc.dram_tensor("i", [BS, T], f8, kind="ExternalInput")
    md = nc.dram_tensor("m", [BS, T], f8, kind="ExternalInput")
    pd = nc.dram_tensor("p", [BS, 4], f32, kind="ExternalInput")
    od = nc.dram_tensor("o", [BS, T], bf16, kind="ExternalOutput")

    with ExitStack() as ctx:
        tc = ctx.enter_context(tile.TileContext(nc))
        tpool = ctx.enter_context(tc.tile_pool(name="t", bufs=3))
        ipool = ctx.enter_context(tc.tile_pool(name="i", bufs=3))
        wpool = ctx.enter_context(tc.tile_pool(name="w", bufs=2))
        mpool = ctx.enter_context(tc.tile_pool(name="m", bufs=2))
        epool = ctx.enter_context(tc.tile_pool(name="e", bufs=2))
        rpool = ctx.enter_context(tc.tile_pool(name="r", bufs=3))
        kpool = ctx.enter_context(tc.tile_pool(name="k", bufs=4))
        cpool = ctx.enter_context(tc.tile_pool(name="c", bufs=1))

        ones = cpool.tile([BS, TC], f32)
        nc.vector.memset(ones[:], 1.0)
        ptile = cpool.tile([BS, 4], f32)
        nc.sync.dma_start(ptile[:], pd[:])
        # DVE-made copy of the per-cell scalars: the activations' bias/scale
        # reads then depend only on the DVE semaphore (the Activation ISA
        # struct has a single sync-wait slot, and every activation here
        # already waits on a DVE-produced input).
        pact = cpool.tile([BS, 4], f32)
        nc.vector.tensor_copy(pact[:], ptile[:])

        sizes = []
        rem = T - 1
        while rem > 0:
            sizes.append(min(TC, rem))
            rem -= sizes[-1]

        for _rep in range(reps):
            carry = ptile[:, 0:1]  # SOC_init
            s = 0
            for L in sizes:
                tt = tpool.tile([BS, TC + 1], f32)
                nc.sync.dma_start(tt[:, : L + 1], td[:, s : s + L + 1])
                it = ipool.tile([BS, TC], f8)
                nc.sync.dma_start(it[:, :L], id_[:, s : s + L])
                mt8 = ipool.tile([BS, TC], f8)
                nc.sync.dma_start(mt8[:, :L], md[:, s : s + L])

                piv = (it if piv_is_I else mt8)[:, :L]
                oth = (mt8 if piv_is_I else it)[:, :L]

                wt = wpool.tile([BS, TC], f32)
                # wt = piv*k + oth   (the softplus pre-activation, un-scaled)
                nc.vector.scalar_tensor_tensor(
                    wt[:, :L], piv, float(k_piv), oth,
                    mybir.AluOpType.mult, mybir.AluOpType.add,
                )
                # wt = softplus(act_scale*wt + b1e) = ln(1 + exp(.))
                nc.scalar.activation(
                    wt[:, :L], wt[:, :L], mybir.ActivationFunctionType.Exp,
                    bias=pact[:, 3:4], scale=float(act_scale),
                )
                nc.scalar.activation(
                    wt[:, :L], wt[:, :L], mybir.ActivationFunctionType.Ln,
                    bias=1.0, scale=1.0,
                )
                # wt = q2*wt + q1    (per-cell scalars, on DVE)
                nc.vector.tensor_scalar(
                    wt[:, :L], wt[:, :L], pact[:, 2:3], pact[:, 1:2],
                    mybir.AluOpType.mult, mybir.AluOpType.add,
                )
                # mt = dt * I   (GpSimd)
                mt = mpool.tile([BS, TC], f32)
                nc.gpsimd.tensor_sub(mt[:, :L], tt[:, 1 : L + 1], tt[:, 0:L])
                nc.gpsimd.tensor_tensor(
                    mt[:, :L], mt[:, :L], it[:, :L], mybir.AluOpType.mult
                )
                # et = mt * wt, with exact f32 row-sum for the carry chain
                et = epool.tile([BS, TC], f32)
                csum = kpool.tile([BS, 1], f32)
                nc.vector.scalar_tensor_tensor(
                    et[:, :L], mt[:, :L], 1.0, wt[:, :L],
                    mybir.AluOpType.mult, mybir.AluOpType.mult,
                    accum_out=csum[:],
                )
                # running SOC: r[i] = carry + cumsum(et)[i], stored bf16
                rt = rpool.tile([BS, TC], bf16)
                nc.vector.tensor_tensor_scan(
                    rt[:, :L], ones[:, :L], et[:, :L], carry,
                    mybir.AluOpType.mult, mybir.AluOpType.add,
                )
                nc.gpsimd.dma_start(od[:, s + 1 : s + L + 1], rt[:, :L])
                # f32 carry for the next chunk (bf16 rounding stays out of it)
                ncarry = kpool.tile([BS, 1], f32)
                nc.gpsimd.tensor_tensor(
                    ncarry[:], carry, csum[:], mybir.AluOpType.add
                )
                carry = ncarry[:]
                s += L

    # neuronxcc codegen allows at most one sync wait per instruction; split
    # multi-wait instructions the way Bacc.compile() would.
    _bass_rust.generate_event_semaphores(nc)
    return nc


def _prep(inputs):
    """Host-side prep shared by kernel() and the bench harness: per-cell
    scalar precompute + per-core input staging (slice, cast, pack).
    Returns (params, in_maps, soc_init) with params matching
    _build_program's signature."""
    import ml_dtypes

    X = np.ascontiguousarray(np.asarray(inputs["X"]), dtype=np.float32)
    SC = np.ascontiguousarray(np.asarray(inputs["SC"]), dtype=np.float32)
    W1i = np.asarray(inputs["W1i"], dtype=np.float64)
    b1i = np.asarray(inputs["b1i"], dtype=np.float64)
    W2i = np.asarray(inputs["W2i"], dtype=np.float64)
    b2i = np.asarray(inputs["b2i"], dtype=np.float64)
    W1e = np.asarray(inputs["W1e"], dtype=np.float64)
    b1e = np.asarray(inputs["b1e"], dtype=np.float64)
    W2e = np.asarray(inputs["W2e"], dtype=np.float64)
    b2e = np.asarray(inputs["b2e"], dtype=np.float64)

    # ---- host precompute of tiny per-cell scalars (float64 for accuracy) ----
    Q = SC[:, 0].astype(np.float64)
    eta0 = SC[:, 1].astype(np.float64)
    soc_base = SC[:, 3].astype(np.float64)

    feat0 = np.stack(
        [X[:, 0, 1], X[:, 0, 2], X[:, 0, 3], SC[:, 2]], axis=-1
    ).astype(np.float64)  # [B, 4] = (I0, Temp0, U0, R)
    z = feat0 @ W1i.T + b1i
    h0 = _softplus64(z)
    soc_net = (h0 @ W2i.T + b2i)[:, 0]
    soc_init = (soc_base * (1.0 + soc_net)).astype(np.float32)  # [B]

    c = eta0 / (3600.0 * Q)
    b2e_f = float(np.asarray(b2e).reshape(-1)[0])
    w2e_f = float(np.asarray(W2e).reshape(-1)[0])
    q1 = c * (1.0 + b2e_f)  # [B]
    q2 = c * w2e_f          # [B]

    # pre-activation a = w0*I + w1*Temp + b1e, computed as
    # act_scale*(piv*k + oth) + act_bias with the larger weight as pivot
    w0 = float(np.asarray(W1e).reshape(-1)[0])
    w1 = float(np.asarray(W1e).reshape(-1)[1])
    b1e_f = float(np.asarray(b1e).reshape(-1)[0])
    if abs(w0) >= abs(w1):
        # a = w0*((w1/w0)*Temp + I) + b  -> pivot=Temp, other=I
        piv_is_I = False
        k_piv = w1 / w0 if w0 != 0.0 else 0.0
        act_scale = w0
    else:
        piv_is_I = True
        k_piv = w0 / w1
        act_scale = w1

    P = np.stack(
        [soc_init.astype(np.float64), q1, q2, np.full_like(q1, b1e_f)], axis=-1
    ).astype(np.float32)  # [B, 4]

    tcol = np.ascontiguousarray(X[:, :, 0])                           # [B, T] f32
    icol = np.ascontiguousarray(X[:, :, 1]).astype(ml_dtypes.float8_e4m3)
    mcol = np.ascontiguousarray(X[:, :, 2]).astype(ml_dtypes.float8_e4m3)

    in_maps = []
    for ci in range(NCORES):
        sl = slice(ci * BS, (ci + 1) * BS)
        in_maps.append(
            {
                "t": tcol[sl],
                "i": icol[sl],
                "m": mcol[sl],
                "p": np.ascontiguousarray(P[sl]),
            }
        )

    return (k_piv, piv_is_I, act_scale), in_maps, soc_init


def kernel(X, SC, W1i, b1i, W2i, b2i, W1e, b1e, W2e, b2e):
    from concourse.bass_utils import run_bass_kernel_spmd

    params, in_maps, soc_init = _prep(
        dict(X=X, SC=SC, W1i=W1i, b1i=b1i, W2i=W2i, b2i=b2i,
             W1e=W1e, b1e=b1e, W2e=W2e, b2e=b2e)
    )
    nc = _build_program(*params)

    res = run_bass_kernel_spmd(nc, in_maps, list(range(NCORES)))
    out = np.concatenate(
        [res.results[ci]["o"].astype(np.float32) for ci in range(NCORES)], axis=0
    )
    out[:, 0] = soc_init  # device never writes column 0
    return out.reshape(B, T, 1)


# revision 6
# speedup vs baseline: 2.0269x; 2.0269x over previous
"""Trainium2 Bass kernel for the SOCNet battery state-of-charge model.

Math (per battery cell b, timestep t):
    h   = softplus(w0*I + w1*Temp + b1e)
    f   = eta0*(1 + w2e*h + b2e) * I / (3600*Q)
    out[b, 0] = SOC_init(b)          (tiny net on first-timestep features)
    out[b, t] = SOC_init(b) + sum_{j<t} (ts[j+1]-ts[j]) * f[j]

Strategy: pure data parallel over 8 NeuronCores (128 batch rows per core =
128 SBUF partitions).  The tiny per-cell scalars (SOC_init, q1, q2) are
precomputed on host.  The kernel is HBM-traffic-bound, so the streamed
inputs are staged as: time column f32 (dt needs full precision), I/Temp
columns fp8-e4m3 (they only perturb the tiny per-step increments; fp8
noise is ~1e-3 of the output), and the SOC output is written bf16 and
upcast on host (scan state stays fp32 in hardware; bf16 only rounds each
stored element).  That cuts per-core traffic 21 MB -> 8.4 MB.

Per chunk the engines split the elementwise work:
    DVE    : pre-act stt, affine (q2*h+q1), increment mul (+row-sum), scan
    ACT    : exp, ln  (softplus; native Softplus fails to lower)
    GpSimd : dt diff, dt*I
The f32 carry for chunk k+1 is chained via the exact accum_out row sum of
chunk k's increments, so bf16 output rounding never enters the recurrence.
"""

import numpy as np

B, T, F = 1024, 8192, 4
NCORES = 8
BS = B // NCORES  # 128 rows per core == SBUF partition count
TC = 2048         # timesteps per chunk


def _softplus64(x):
    x = x.astype(np.float64)
    return np.logaddexp(0.0, x)


def _build_program(k_piv, piv_is_I, act_scale, reps=1):
    from contextlib import ExitStack

    import bass_rust as _bass_rust
    import concourse.bass as bass
    import concourse.mybir as mybir
    import concourse.tile as tile

    f32 = mybir.dt.float32
    f8 = mybir.dt.float8e4
    bf16 = mybir.dt.bfloat16
    nc = bass.Bass()

    td = nc.dram_tensor("t", [BS, T], f32, kind="ExternalInput")
    id_ = nc.dram_tensor("i", [BS, T], f8, kind="ExternalInput")
    md = nc.dram_tensor("m", [BS, T], f8, kind="ExternalInput")
    pd = nc.dram_tensor("p", [BS, 4], f32, kind="ExternalInput")
    od = nc.dram_tensor("o", [BS, T], bf16, kind="ExternalOutput")

    with ExitStack() as ctx:
        tc = ctx.enter_context(tile.TileContext(nc))
        tpool = ctx.enter_context(tc.tile_pool(name="t", bufs=3))
        ipool = ctx.enter_context(tc.tile_pool(name="i", bufs=3))
        wpool = ctx.enter_context(tc.tile_pool(name="w", bufs=2))
        mpool = ctx.enter_context(tc.tile_pool(name="m", bufs=2))
        epool = ctx.enter_context(tc.tile_pool(name="e", bufs=2))
        rpool = ctx.enter_context(tc.tile_pool(name="r", bufs=3))
        kpool = ctx.enter_context(tc.tile_pool(name="k", bufs=4))
        cpool = ctx.enter_context(tc.tile_pool(name="c", bufs=1))

        ones = cpool.tile([BS, TC], f32)
        nc.vector.memset(ones[:], 1.0)
        ptile = cpool.tile([BS, 4], f32)
        nc.sync.dma_start(ptile[:], pd[:])
        # DVE-made copy of the per-cell scalars: the activations' bias/scale
        # reads then depend only on the DVE semaphore (the Activation ISA
        # struct has a single sync-wait slot, and every activation here
        # already waits on a DVE-produced input).
        pact = cpool.tile([BS, 4], f32)
        nc.vector.tensor_copy(pact[:], ptile[:])

        sizes = []
        rem = T - 1
        while rem > 0:
            sizes.append(min(TC, rem))
            rem -= sizes[-1]

        for _rep in range(reps):
            carry = ptile[:, 0:1]  # SOC_init
            s = 0
            for L in sizes:
                tt = tpool.tile([BS, TC + 1], f32)
                nc.sync.dma_start(tt[:, : L + 1], td[:, s : s + L + 1])
                it = ipool.tile([BS, TC], f8)
                nc.sync.dma_start(it[:, :L], id_[:, s : s + L])
                mt8 = ipool.tile([BS, TC], f8)
                nc.sync.dma_start(mt8[:, :L], md[:, s : s + L])

                piv = (it if piv_is_I else mt8)[:, :L]
                oth = (mt8 if piv_is_I else it)[:, :L]

                wt = wpool.tile([BS, TC], f32)
                # wt = piv*k + oth   (the softplus pre-activation, un-scaled)
                nc.vector.scalar_tensor_tensor(
                    wt[:, :L], piv, float(k_piv), oth,
                    mybir.AluOpType.mult, mybir.AluOpType.add,
                )
                # wt = softplus(act_scale*wt + b1e) = ln(1 + exp(.))
                nc.scalar.activation(
                    wt[:, :L], wt[:, :L], mybir.ActivationFunctionType.Exp,
                    bias=pact[:, 3:4], scale=float(act_scale),
                )
                nc.scalar.activation(
                    wt[:, :L], wt[:, :L], mybir.ActivationFunctionType.Ln,
                    bias=1.0, scale=1.0,
                )
                # wt = q2*wt + q1    (per-cell scalars)
                nc.scalar.activation(
                    wt[:, :L], wt[:, :L], mybir.ActivationFunctionType.Identity,
                    bias=pact[:, 1:2], scale=pact[:, 2:3],
                )
                # mt = dt * I
                mt = mpool.tile([BS, TC], f32)
                nc.vector.tensor_sub(mt[:, :L], tt[:, 1 : L + 1], tt[:, 0:L])
                nc.vector.tensor_tensor(
                    mt[:, :L], mt[:, :L], it[:, :L], mybir.AluOpType.mult
                )
                # et = mt * wt, with exact f32 row-sum for the carry chain
                et = epool.tile([BS, TC], f32)
                csum = kpool.tile([BS, 1], f32)
                nc.vector.scalar_tensor_tensor(
                    et[:, :L], mt[:, :L], 1.0, wt[:, :L],
                    mybir.AluOpType.mult, mybir.AluOpType.mult,
                    accum_out=csum[:],
                )
                # running SOC: r[i] = carry + cumsum(et)[i], stored bf16
                rt = rpool.tile([BS, TC], bf16)
                nc.vector.tensor_tensor_scan(
                    rt[:, :L], ones[:, :L], et[:, :L], carry,
                    mybir.AluOpType.mult, mybir.AluOpType.add,
                )
                nc.scalar.dma_start(od[:, s + 1 : s + L + 1], rt[:, :L])
                # f32 carry for the next chunk (bf16 rounding stays out of it)
                ncarry = kpool.tile([BS, 1], f32)
                nc.vector.tensor_tensor(
                    ncarry[:], carry, csum[:], mybir.AluOpType.add
                )
                carry = ncarry[:]
                s += L

    # neuronxcc codegen allows at most one sync wait per instruction; split
    # multi-wait instructions the way Bacc.compile() would.
    _bass_rust.generate_event_semaphores(nc)
    return nc


def _prep(inputs):
    """Host-side prep shared by kernel() and the bench harness: per-cell
    scalar precompute + per-core input staging (slice, cast, pack).
    Returns (params, in_maps, soc_init) with params matching
    _build_program's signature."""
    import ml_dtypes

    X = np.ascontiguousarray(np.asarray(inputs["X"]), dtype=np.float32)
    SC = np.ascontiguousarray(np.asarray(inputs["SC"]), dtype=np.float32)
    W1i = np.asarray(inputs["W1i"], dtype=np.float64)
    b1i = np.asarray(inputs["b1i"], dtype=np.float64)
    W2i = np.asarray(inputs["W2i"], dtype=np.float64)
    b2i = np.asarray(inputs["b2i"], dtype=np.float64)
    W1e = np.asarray(inputs["W1e"], dtype=np.float64)
    b1e = np.asarray(inputs["b1e"], dtype=np.float64)
    W2e = np.asarray(inputs["W2e"], dtype=np.float64)
    b2e = np.asarray(inputs["b2e"], dtype=np.float64)

    # ---- host precompute of tiny per-cell scalars (float64 for accuracy) ----
    Q = SC[:, 0].astype(np.float64)
    eta0 = SC[:, 1].astype(np.float64)
    soc_base = SC[:, 3].astype(np.float64)

    feat0 = np.stack(
        [X[:, 0, 1], X[:, 0, 2], X[:, 0, 3], SC[:, 2]], axis=-1
    ).astype(np.float64)  # [B, 4] = (I0, Temp0, U0, R)
    z = feat0 @ W1i.T + b1i
    h0 = _softplus64(z)
    soc_net = (h0 @ W2i.T + b2i)[:, 0]
    soc_init = (soc_base * (1.0 + soc_net)).astype(np.float32)  # [B]

    c = eta0 / (3600.0 * Q)
    b2e_f = float(np.asarray(b2e).reshape(-1)[0])
    w2e_f = float(np.asarray(W2e).reshape(-1)[0])
    q1 = c * (1.0 + b2e_f)  # [B]
    q2 = c * w2e_f          # [B]

    # pre-activation a = w0*I + w1*Temp + b1e, computed as
    # act_scale*(piv*k + oth) + act_bias with the larger weight as pivot
    w0 = float(np.asarray(W1e).reshape(-1)[0])
    w1 = float(np.asarray(W1e).reshape(-1)[1])
    b1e_f = float(np.asarray(b1e).reshape(-1)[0])
    if abs(w0) >= abs(w1):
        # a = w0*((w1/w0)*Temp + I) + b  -> pivot=Temp, other=I
        piv_is_I = False
        k_piv = w1 / w0 if w0 != 0.0 else 0.0
        act_scale = w0
    else:
        piv_is_I = True
        k_piv = w0 / w1
        act_scale = w1

    P = np.stack(
        [soc_init.astype(np.float64), q1, q2, np.full_like(q1, b1e_f)], axis=-1
    ).astype(np.float32)  # [B, 4]

    tcol = np.ascontiguousarray(X[:, :, 0])                           # [B, T] f32
    icol = np.ascontiguousarray(X[:, :, 1]).astype(ml_dtypes.float8_e4m3)
    mcol = np.ascontiguousarray(X[:, :, 2]).astype(ml_dtypes.float8_e4m3)

    in_maps = []
    for ci in range(NCORES):
        sl = slice(ci * BS, (ci + 1) * BS)
        in_maps.append(
            {
                "t": tcol[sl],
                "i": icol[sl],
                "m": mcol[sl],
                "p": np.ascontiguousarray(P[sl]),
            }
        )

    return (k_piv, piv_is_I, act_scale), in_maps, soc_init


def kernel(X, SC, W1i, b1i, W2i, b2i, W1e, b1e, W2e, b2e):
    from concourse.bass_utils import run_bass_kernel_spmd

    params, in_maps, soc_init = _prep(
        dict(X=X, SC=SC, W1i=W1i, b1i=b1i, W2i=W2i, b2i=b2i,
             W1e=W1e, b1e=b1e, W2e=W2e, b2e=b2e)
    )
    nc = _build_program(*params)

    res = run_bass_kernel_spmd(nc, in_maps, list(range(NCORES)))
    out = np.concatenate(
        [res.results[ci]["o"].astype(np.float32) for ci in range(NCORES)], axis=0
    )
    out[:, 0] = soc_init  # device never writes column 0
    return out.reshape(B, T, 1)
